# revision 17
# baseline (speedup 1.0000x reference)
"""DGCNN Trainium2 kernel: 16 point clouds sharded 2-per-core across 8 NeuronCores.

Decomposition per edge_conv layer (BN folded into weights on host):
  h[o,n,k] = Wn'@x_j + (Wc'-Wn')@x_n + b'   (j = k-th neighbor of n)
  out[o,n] = max_k LReLU(h) = LReLU(max_k u[o,j] + v[o,n])   (LReLU monotone,
             BN scale > 0), with u = Wn'@x, v = (Wc'-Wn')@x + b'.
kNN ranking per row n uses s[n,m] = x_n.x_m - 0.5*||x_m||^2 (rank-equivalent
to -||x_n - x_m||^2; self is always rank 1, matching jax top_k on the
reference's pairwise matrix).
"""
import sys
sys.path.insert(0, '/opt/trn_rl_repo')
import numpy as np
import concourse.bass as bass
import concourse.bacc as bacc
import concourse.mybir as mybir
import concourse.tile as tile
from concourse import bass_utils

F32 = mybir.dt.float32
F32R = mybir.dt.float32r
U32 = mybir.dt.uint32
AF = mybir.ActivationFunctionType
ALU = mybir.AluOpType
AX = mybir.AxisListType

N = 2048
K = 20
P = 128
NT = N // P                 # 16 row tiles
BPC = 2                     # clouds per core
NCORES = 8
E5 = 1024                   # final embedding dim
CAT = 512
# (Cin, Cout, cat_offset) per edge layer
LAYERS = [(3, 64, 0), (64, 64, 64), (64, 128, 128), (128, 256, 256)]
NEG = -3.0e38

_DBG = None  # when set: dict to register debug DRAM dumps


def _dump(nc, name, ap):
    if _DBG is None:
        return
    t = nc.dram_tensor(f"dbg_{name}", list(ap.shape), ap.tensor.dtype,
                       kind="ExternalOutput")
    nc.sync.dma_start(out=t[:], in_=ap)
    _DBG[name] = list(ap.shape)


def build_program(use_f32r=False, debug=False):
    global _DBG
    _DBG = {} if debug else None
    nc = bacc.Bacc("TRN2", target_bir_lowering=False, debug=False)

    xin = nc.dram_tensor("xin", [BPC, 4, N], F32, kind="ExternalInput")  # rows 0..2=x, 3=pad
    wn = [nc.dram_tensor(f"wn{l}", [c, o], F32, kind="ExternalInput")
          for l, (c, o, _) in enumerate(LAYERS)]
    wv = [nc.dram_tensor(f"wv{l}", [c + 1, o], F32, kind="ExternalInput")
          for l, (c, o, _) in enumerate(LAYERS)]
    w5 = nc.dram_tensor("w5", [CAT, E5], F32, kind="ExternalInput")
    b5 = nc.dram_tensor("b5", [1, E5], F32, kind="ExternalInput")
    ident = nc.dram_tensor("ident", [P, P], F32, kind="ExternalInput")
    out = nc.dram_tensor("out", [BPC, 2 * E5], F32, kind="ExternalOutput")

    def mmcast(ap):
        return ap.bitcast(F32R) if use_f32r else ap

    with tile.TileContext(nc) as tc:
        with tc.tile_pool(name="const", bufs=1) as cpool, \
             tc.tile_pool(name="dram", bufs=1, space="DRAM") as dpool:
            ident_t = cpool.tile([P, P], F32, tag="ident")
            nc.sync.dma_start(out=ident_t[:], in_=ident[:])
            ones_row = cpool.tile([1, N], F32, tag="ones")
            nc.vector.memset(ones_row[:], 1.0)
            neghalf = cpool.tile([P, 1], F32, tag="neghalf")
            nc.vector.memset(neghalf[:], -0.5)
            wn_t = []
            wv_t = []
            for l, (c, o, _) in enumerate(LAYERS):
                wnt = cpool.tile([c, o], F32, tag=f"wn{l}")
                nc.sync.dma_start(out=wnt[:], in_=wn[l][:])
                wva = cpool.tile([min(c + 1, P), o], F32, tag=f"wv{l}")
                nc.sync.dma_start(out=wva[:], in_=wv[l][0:min(c + 1, P), :])
                wvb = None
                if c + 1 > P:  # layer 4: bias row separate
                    wvb = cpool.tile([1, o], F32, tag=f"wvb{l}")
                    nc.sync.dma_start(out=wvb[:], in_=wv[l][c:c + 1, :])
                wn_t.append(wnt)
                wv_t.append((wva, wvb))
            w5_t = [cpool.tile([P, E5], F32, tag=f"w5_{cc}", name=f"w5_{cc}") for cc in range(4)]
            for cc in range(4):
                nc.sync.dma_start(out=w5_t[cc][:], in_=w5[cc * P:(cc + 1) * P, :])
            b5_t = cpool.tile([1, E5], F32, tag="b5")
            nc.sync.dma_start(out=b5_t[:], in_=b5[:])

            u_dram = [dpool.tile([N, o], F32, tag=f"u{l}_{c}", name=f"u{l}_{c}")
                      for c in range(BPC) for l, (_, o, _) in enumerate(LAYERS)]

            for cloud in range(BPC):
                emit_cloud(nc, tc, cloud, xin, out, ident_t, ones_row, neghalf,
                           wn_t, wv_t, w5_t, b5_t,
                           u_dram[cloud * 4:(cloud + 1) * 4], mmcast)

    nc.compile()
    return nc


def emit_cloud(nc, tc, cloud, xin, out, ident_t, ones_row, neghalf,
               wn_t, wv_t, w5_t, b5_t, u_dram, mmcast):
    with tc.tile_pool(name=f"cl{cloud}", bufs=1) as per, \
         tc.tile_pool(name=f"cl{cloud}w", bufs=2) as wrk, \
         tc.tile_pool(name=f"cl{cloud}ps", bufs=1, space="PSUM") as psp, \
         tc.tile_pool(name=f"cl{cloud}ps2", bufs=2, space="PSUM") as psp2:
        cat_t = [per.tile([P, N], F32, tag=f"cat{cc}", name=f"cat{cloud}_{cc}") for cc in range(4)]
        x_aug = per.tile([P, N], F32, tag="x_aug")
        x_rhs = per.tile([P, N], F32, tag="x_rhs")
        xxn_row = per.tile([1, N], F32, tag="xxn_row")

        # L1 input: x (3, N) + ones row at row 3
        nc.sync.dma_start(out=x_aug[0:4, :], in_=xin[cloud])
        nc.sync.dma_start(out=x_rhs[0:3, :], in_=x_aug[0:3, :])

        for l, (C, O, catofs) in enumerate(LAYERS):
            aug = C + 1 <= P  # ones/xx row fits inside the 128-row tiles
            emit_edge_layer(nc, tc, per, wrk, psp, psp2, l, C, O, catofs, aug,
                            cat_t, x_aug, x_rhs, xxn_row, ones_row, neghalf,
                            ident_t, wn_t[l], wv_t[l], u_dram[l], mmcast)
            if l < 3:
                Cn = LAYERS[l + 1][0]
                ofs = LAYERS[l][2]  # this layer's output rows start at its catofs
                src = cat_t[ofs // P]
                nc.sync.dma_start(out=x_aug[0:Cn, :], in_=src[ofs % P:ofs % P + Cn, :])
                if Cn + 1 <= P:
                    nc.vector.memset(x_aug[Cn:Cn + 1, :], 1.0)
                nc.sync.dma_start(out=x_rhs[0:Cn, :], in_=x_aug[0:Cn, :])

        emit_final(nc, tc, wrk, psp2, cloud, out, cat_t, w5_t, b5_t, ones_row,
                   mmcast)


def emit_edge_layer(nc, tc, per, wrk, psp, psp2, l, C, O, catofs, aug,
                    cat_t, x_aug, x_rhs, xxn_row, ones_row, neghalf,
                    ident_t, wn_l, wv_l, u_dram_l, mmcast):
    wva, wvb = wv_l

    # ---- xx prep: x_rhs row C (or xxn_row) = -0.5 * sum_c x^2 ----
    sq = wrk.tile([P, N], F32, tag="big")
    nc.scalar.activation(sq[0:C, :], x_aug[0:C, :], AF.Square)
    xx_ps = psp.tile([1, 512], F32, tag="small_a", padded_shape=[P, 512])
    direct = aug and C % 32 == 0
    xx_dst = x_rhs[C:C + 1, :] if direct else xxn_row[:]
    for m in range(4):
        nc.tensor.matmul(xx_ps[:], mmcast(neghalf[0:C, :]),
                         mmcast(sq[0:C, m * 512:(m + 1) * 512]),
                         start=True, stop=True)
        nc.scalar.activation(
            bass.AP(xx_dst.tensor, xx_dst.offset + m * 512,
                    [xx_dst.ap[0], [1, 512]]),
            xx_ps[:], AF.Copy)
    if aug and not direct:
        nc.sync.dma_start(out=x_rhs[C:C + 1, :], in_=xxn_row[:])

    idx_all = per.tile([P, NT * 24], U32, tag="idx_all")
    v_all = per.tile([P, NT * O], F32, tag="v_all", padded_shape=[P, 16 * 256])

    # ---- phase 1: pairwise + selection + u/v per row tile ----
    for t in range(NT):
        s_ps = psp.tile([P, N], F32, tag="s")
        lhsT = x_aug[0:C + (1 if aug else 0), t * P:(t + 1) * P]
        for m in range(4):
            rhs = x_rhs[0:C + (1 if aug else 0), m * 512:(m + 1) * 512]
            nc.tensor.matmul(s_ps[:, m * 512:(m + 1) * 512], mmcast(lhsT),
                             mmcast(rhs), start=True, stop=aug)
            if not aug:
                nc.tensor.matmul(s_ps[:, m * 512:(m + 1) * 512],
                                 mmcast(ones_row[0:1, t * P:(t + 1) * P]),
                                 mmcast(xxn_row[0:1, m * 512:(m + 1) * 512]),
                                 start=False, stop=True)
        # s -> SBUF (ACT), then 3 rounds of top-8 extraction (DVE)
        s_sb = wrk.tile([P, N], F32, tag="big")
        nc.scalar.activation(s_sb[:], s_ps[:], AF.Copy)
        if t == 0:
            _dump(nc, f"s_l{l}_{cat_t[0].tensor.name}", s_sb[:])
        v8 = wrk.tile([P, 8], F32, tag="v8")
        for r in range(3):
            nc.vector.max(v8[:], s_sb[:])
            nc.vector.max_index(idx_all[:, t * 24 + r * 8:t * 24 + r * 8 + 8],
                                v8[:], s_sb[:])
            if r < 2:
                nc.vector.match_replace(s_sb[:], v8[:], s_sb[:], NEG)

        # u^T tile -> DRAM ; v^T tile -> v_all
        u_ps = psp.tile([P, O], F32, tag="small_a", padded_shape=[P, 512])
        nc.tensor.matmul(u_ps[:], mmcast(x_aug[0:C, t * P:(t + 1) * P]),
                         mmcast(wn_l[:]), start=True, stop=True)
        u_sb = wrk.tile([P, O], F32, tag="u_sb", padded_shape=[P, 256])
        nc.scalar.activation(u_sb[:], u_ps[:], AF.Copy)
        nc.sync.dma_start(out=u_dram_l[t * P:(t + 1) * P, :], in_=u_sb[:])
        v_ps = psp.tile([P, O], F32, tag="small_b", padded_shape=[P, 512])
        if aug:
            nc.tensor.matmul(v_ps[:], mmcast(x_aug[0:C + 1, t * P:(t + 1) * P]),
                             mmcast(wva[:]), start=True, stop=True)
        else:
            nc.tensor.matmul(v_ps[:], mmcast(x_aug[0:C, t * P:(t + 1) * P]),
                             mmcast(wva[:]), start=True, stop=False)
            nc.tensor.matmul(v_ps[:], mmcast(ones_row[0:1, t * P:(t + 1) * P]),
                             mmcast(wvb[:]), start=False, stop=True)
        nc.scalar.activation(v_all[:, t * O:(t + 1) * O], v_ps[:], AF.Copy)

    _dump(nc, f"idx_l{l}_{cat_t[0].tensor.name}", idx_all[:])
    _dump(nc, f"v_l{l}_{cat_t[0].tensor.name}", v_all[:, 0:NT * O])

    # ---- phase 2: gather u rows of the 20 nearest, combine, transpose ----
    for t in range(NT):
        gath = wrk.tile([P, K, O], F32, tag="gath", padded_shape=[P, 20, 256])
        for kk in range(K):
            nc.gpsimd.indirect_dma_start(
                out=gath[:, kk, :], out_offset=None,
                in_=u_dram_l[:],
                in_offset=bass.IndirectOffsetOnAxis(
                    ap=idx_all[:, t * 24 + kk:t * 24 + kk + 1], axis=0))
        xn = wrk.tile([P, O], F32, tag="xn", padded_shape=[P, 256])
        nc.vector.tensor_reduce(out=xn[:], in_=gath[:].rearrange("p k c -> p c k"),
                                axis=AX.X, op=ALU.max)
        nc.vector.tensor_tensor(out=xn[:], in0=xn[:],
                                in1=v_all[:, t * O:(t + 1) * O], op=ALU.add)
        xn2 = wrk.tile([P, O], F32, tag="xn2", padded_shape=[P, 256])
        nc.vector.tensor_scalar(out=xn2[:], in0=xn[:], scalar1=0.2, scalar2=None,
                                op0=ALU.mult)
        nc.vector.tensor_tensor(out=xn[:], in0=xn[:], in1=xn2[:], op=ALU.max)
        if t == 0:
            _dump(nc, f"gath_l{l}_{cat_t[0].tensor.name}", gath[:])
            _dump(nc, f"xn_l{l}_{cat_t[0].tensor.name}", xn[:])
        for blk in range(O // P if O >= P else 1):
            bw = min(P, O)
            tr_ps = psp2.tile([P, P], F32, tag="tb", padded_shape=[P, 512])
            nc.tensor.transpose(out=tr_ps[0:bw, :],
                                in_=xn[:, blk * P:blk * P + bw],
                                identity=ident_t[:])
            row = catofs + blk * P
            nc.scalar.activation(
                cat_t[row // P][row % P:row % P + bw, t * P:(t + 1) * P],
                tr_ps[0:bw, :], AF.Copy)


def emit_final(nc, tc, wrk, psp2, cloud, out, cat_t, w5_t, b5_t, ones_row,
               mmcast):
    # h = LReLU(W5' cat + b5'); out = [max_n h, mean_n h]
    for eb in range(E5 // P):
        gmax = wrk.tile([P, 4], F32, tag="gmax")
        spre = wrk.tile([P, 4], F32, tag="spre")
        srel = wrk.tile([P, 4], F32, tag="srel")
        for m in range(4):
            h_ps = psp2.tile([P, 512], F32, tag="tb")
            for cc in range(4):
                nc.tensor.matmul(h_ps[:], mmcast(w5_t[cc][:, eb * P:(eb + 1) * P]),
                                 mmcast(cat_t[cc][:, m * 512:(m + 1) * 512]),
                                 start=(cc == 0), stop=False)
            nc.tensor.matmul(h_ps[:], mmcast(b5_t[0:1, eb * P:(eb + 1) * P]),
                             mmcast(ones_row[0:1, m * 512:(m + 1) * 512]),
                             start=False, stop=True)
            nc.vector.tensor_reduce(out=gmax[:, m:m + 1], in_=h_ps[:],
                                    axis=AX.X, op=ALU.max)
            scr = wrk.tile([P, 512], F32, tag="scr")
            nc.scalar.activation(scr[:], h_ps[:], AF.Copy,
                                 accum_out=spre[:, m:m + 1])
            nc.scalar.activation(scr[:], h_ps[:], AF.Relu,
                                 accum_out=srel[:, m:m + 1])
        g1 = wrk.tile([P, 1], F32, tag="g1")
        nc.vector.tensor_reduce(out=g1[:], in_=gmax[:], axis=AX.X, op=ALU.max)
        g2 = wrk.tile([P, 1], F32, tag="g2")
        nc.vector.tensor_scalar(out=g2[:], in0=g1[:], scalar1=0.2, scalar2=None,
                                op0=ALU.mult)
        nc.vector.tensor_tensor(out=g1[:], in0=g1[:], in1=g2[:], op=ALU.max)
        nc.sync.dma_start(out=out[cloud:cloud + 1, eb * P:(eb + 1) * P],
                          in_=g1[:])
        # mean: (0.2*sum(pre) + 0.8*sum(relu)) / N
        sp1 = wrk.tile([P, 1], F32, tag="sp1")
        nc.vector.tensor_reduce(out=sp1[:], in_=spre[:], axis=AX.X, op=ALU.add)
        sr1 = wrk.tile([P, 1], F32, tag="sr1")
        nc.vector.tensor_reduce(out=sr1[:], in_=srel[:], axis=AX.X, op=ALU.add)
        nc.vector.tensor_scalar(out=sp1[:], in0=sp1[:], scalar1=0.2 / N,
                                scalar2=None, op0=ALU.mult)
        nc.vector.tensor_scalar(out=sr1[:], in0=sr1[:], scalar1=0.8 / N,
                                scalar2=None, op0=ALU.mult)
        nc.vector.tensor_tensor(out=sp1[:], in0=sp1[:], in1=sr1[:], op=ALU.add)
        nc.sync.dma_start(out=out[cloud:cloud + 1, E5 + eb * P:E5 + (eb + 1) * P],
                          in_=sp1[:])


_PROGRAM = None


def _get_program():
    global _PROGRAM
    if _PROGRAM is None:
        _PROGRAM = build_program()
    return _PROGRAM


def _fold_bn(p):
    EPS = 1e-5
    W = np.asarray(p['W'], np.float32)
    g, b, m, v = [np.asarray(t, np.float32) for t in p['bn']]
    s = (g / np.sqrt(v + EPS)).astype(np.float32)
    return (W * s[:, None]).astype(np.float32), (b - m * s).astype(np.float32)


def kernel(x, params, _trace=False):
    x = np.asarray(x, np.float32)          # (16, 2048, 3)
    B = x.shape[0]
    nc = _get_program()

    common = {'ident': np.eye(P, dtype=np.float32)}
    for l, key in enumerate(['c1', 'c2', 'c3', 'c4']):
        Wf, bf = _fold_bn(params[key])
        C = Wf.shape[1] // 2
        Wn, Wc = Wf[:, :C], Wf[:, C:]
        common[f'wn{l}'] = np.ascontiguousarray(Wn.T)                  # (C, O)
        wvmat = np.concatenate([(Wc - Wn).T, bf[None, :]], 0)          # (C+1, O)
        common[f'wv{l}'] = np.ascontiguousarray(wvmat)
    W5f, b5f = _fold_bn(params['c5'])
    common['w5'] = np.ascontiguousarray(W5f.T)                         # (512, 1024)
    common['b5'] = np.ascontiguousarray(b5f[None, :])

    in_maps = []
    for core in range(NCORES):
        xc = x[core * BPC:(core + 1) * BPC]                            # (2, 2048, 3)
        xt = np.zeros((BPC, 4, N), np.float32)
        xt[:, 0:3, :] = np.transpose(xc, (0, 2, 1))
        xt[:, 3, :] = 1.0
        m = dict(common)
        m['xin'] = xt
        in_maps.append(m)

    res = bass_utils.run_bass_kernel_spmd(nc, in_maps, list(range(NCORES)),
                                          trace=_trace)
    outs = [res.results[i]['out'] for i in range(NCORES)]              # (2, 2048)
    full = np.concatenate(outs, axis=0).astype(np.float32)             # (16, 2048)
    if _trace:
        return full, res
    return full


# revision 18
# speedup vs baseline: 1.1120x; 1.1120x over previous
"""DGCNN Trainium2 kernel: 16 point clouds sharded 2-per-core across 8 NeuronCores.

Decomposition per edge_conv layer (BN folded into weights on host):
  h[o,n,k] = Wn'@x_j + (Wc'-Wn')@x_n + b'   (j = k-th neighbor of n)
  out[o,n] = max_k LReLU(h) = LReLU(max_k u[o,j] + v[o,n])   (LReLU monotone,
             BN scale > 0), with u = Wn'@x, v = (Wc'-Wn')@x + b'.
kNN ranking per row n uses s[n,m] = x_n.x_m - 0.5*||x_m||^2 (rank-equivalent
to -||x_n - x_m||^2; self is always rank 1, matching jax top_k on the
reference's pairwise matrix).
"""
import sys
sys.path.insert(0, '/opt/trn_rl_repo')
import numpy as np
import concourse.bass as bass
import concourse.bacc as bacc
import concourse.mybir as mybir
import concourse.tile as tile
from concourse import bass_utils

F32 = mybir.dt.float32
F32R = mybir.dt.float32r
U32 = mybir.dt.uint32
AF = mybir.ActivationFunctionType
ALU = mybir.AluOpType
AX = mybir.AxisListType

N = 2048
K = 20
P = 128
NT = N // P                 # 16 row tiles
BPC = 2                     # clouds per core
NCORES = 8
E5 = 1024                   # final embedding dim
CAT = 512
# (Cin, Cout, cat_offset) per edge layer
LAYERS = [(3, 64, 0), (64, 64, 64), (64, 128, 128), (128, 256, 256)]
NEG = -3.0e38

_DBG = None  # when set: dict to register debug DRAM dumps


def _dump(nc, name, ap):
    if _DBG is None:
        return
    t = nc.dram_tensor(f"dbg_{name}", list(ap.shape), ap.tensor.dtype,
                       kind="ExternalOutput")
    nc.sync.dma_start(out=t[:], in_=ap)
    _DBG[name] = list(ap.shape)


def build_program(use_f32r=False, debug=False):
    global _DBG
    _DBG = {} if debug else None
    nc = bacc.Bacc("TRN2", target_bir_lowering=False, debug=False)

    xin = nc.dram_tensor("xin", [BPC, 4, N], F32, kind="ExternalInput")  # rows 0..2=x, 3=pad
    wn = [nc.dram_tensor(f"wn{l}", [c, o], F32, kind="ExternalInput")
          for l, (c, o, _) in enumerate(LAYERS)]
    wv = [nc.dram_tensor(f"wv{l}", [c + 1, o], F32, kind="ExternalInput")
          for l, (c, o, _) in enumerate(LAYERS)]
    w5 = nc.dram_tensor("w5", [CAT, E5], F32, kind="ExternalInput")
    b5 = nc.dram_tensor("b5", [1, E5], F32, kind="ExternalInput")
    ident = nc.dram_tensor("ident", [P, P], F32, kind="ExternalInput")
    out = nc.dram_tensor("out", [BPC, 2 * E5], F32, kind="ExternalOutput")

    def mmcast(ap):
        return ap.bitcast(F32R) if use_f32r else ap

    with tile.TileContext(nc) as tc:
        with tc.tile_pool(name="const", bufs=1) as cpool, \
             tc.tile_pool(name="dram", bufs=1, space="DRAM") as dpool:
            ident_t = cpool.tile([P, P], F32, tag="ident")
            nc.sync.dma_start(out=ident_t[:], in_=ident[:])
            ones_row = cpool.tile([1, N], F32, tag="ones")
            nc.vector.memset(ones_row[:], 1.0)
            neghalf = cpool.tile([P, 1], F32, tag="neghalf")
            nc.vector.memset(neghalf[:], -0.5)
            wn_t = []
            wv_t = []
            for l, (c, o, _) in enumerate(LAYERS):
                wnt = cpool.tile([c, o], F32, tag=f"wn{l}")
                nc.sync.dma_start(out=wnt[:], in_=wn[l][:])
                wva = cpool.tile([min(c + 1, P), o], F32, tag=f"wv{l}")
                nc.sync.dma_start(out=wva[:], in_=wv[l][0:min(c + 1, P), :])
                wvb = None
                if c + 1 > P:  # layer 4: bias row separate
                    wvb = cpool.tile([1, o], F32, tag=f"wvb{l}")
                    nc.sync.dma_start(out=wvb[:], in_=wv[l][c:c + 1, :])
                wn_t.append(wnt)
                wv_t.append((wva, wvb))
            w5_t = [cpool.tile([P, E5], F32, tag=f"w5_{cc}", name=f"w5_{cc}") for cc in range(4)]
            for cc in range(4):
                nc.sync.dma_start(out=w5_t[cc][:], in_=w5[cc * P:(cc + 1) * P, :])
            b5_t = cpool.tile([1, E5], F32, tag="b5")
            nc.sync.dma_start(out=b5_t[:], in_=b5[:])

            u_dram = [dpool.tile([N, o], F32, tag=f"u{l}_{c}", name=f"u{l}_{c}")
                      for c in range(BPC) for l, (_, o, _) in enumerate(LAYERS)]

            for cloud in range(BPC):
                emit_cloud(nc, tc, cloud, xin, out, ident_t, ones_row, neghalf,
                           wn_t, wv_t, w5_t, b5_t,
                           u_dram[cloud * 4:(cloud + 1) * 4], mmcast)

    nc.compile()
    return nc


def emit_cloud(nc, tc, cloud, xin, out, ident_t, ones_row, neghalf,
               wn_t, wv_t, w5_t, b5_t, u_dram, mmcast):
    with tc.tile_pool(name=f"cl{cloud}", bufs=1) as per, \
         tc.tile_pool(name=f"cl{cloud}w", bufs=2) as wrk, \
         tc.tile_pool(name=f"cl{cloud}ps", bufs=1, space="PSUM") as psp, \
         tc.tile_pool(name=f"cl{cloud}ps2", bufs=2, space="PSUM") as psp2:
        cat_t = [per.tile([P, N], F32, tag=f"cat{cc}", name=f"cat{cloud}_{cc}") for cc in range(4)]
        x_aug = per.tile([P, N], F32, tag="x_aug")
        x_rhs = per.tile([P, N], F32, tag="x_rhs")
        xxn_row = per.tile([1, N], F32, tag="xxn_row")

        # L1 input: x (3, N) + ones row at row 3
        nc.sync.dma_start(out=x_aug[0:4, :], in_=xin[cloud])
        nc.sync.dma_start(out=x_rhs[0:3, :], in_=x_aug[0:3, :])

        for l, (C, O, catofs) in enumerate(LAYERS):
            aug = C + 1 <= P  # ones/xx row fits inside the 128-row tiles
            emit_edge_layer(nc, tc, per, wrk, psp, psp2, l, C, O, catofs, aug,
                            cat_t, x_aug, x_rhs, xxn_row, ones_row, neghalf,
                            ident_t, wn_t[l], wv_t[l], u_dram[l], mmcast)
            if l < 3:
                Cn = LAYERS[l + 1][0]
                ofs = LAYERS[l][2]  # this layer's output rows start at its catofs
                src = cat_t[ofs // P]
                nc.sync.dma_start(out=x_aug[0:Cn, :], in_=src[ofs % P:ofs % P + Cn, :])
                if Cn + 1 <= P:
                    nc.vector.memset(x_aug[Cn:Cn + 1, :], 1.0)
                nc.sync.dma_start(out=x_rhs[0:Cn, :], in_=x_aug[0:Cn, :])

        emit_final(nc, tc, wrk, psp2, cloud, out, cat_t, w5_t, b5_t, ones_row,
                   mmcast)


def emit_edge_layer(nc, tc, per, wrk, psp, psp2, l, C, O, catofs, aug,
                    cat_t, x_aug, x_rhs, xxn_row, ones_row, neghalf,
                    ident_t, wn_l, wv_l, u_dram_l, mmcast):
    wva, wvb = wv_l

    # ---- xx prep: x_rhs row C (or xxn_row) = -0.5 * sum_c x^2 ----
    sq = wrk.tile([P, N], F32, tag="big", bufs=3)
    nc.scalar.activation(sq[0:C, :], x_aug[0:C, :], AF.Square)
    xx_ps = psp.tile([1, 512], F32, tag="small_a", padded_shape=[P, 512])
    direct = aug and C % 32 == 0
    xx_dst = x_rhs[C:C + 1, :] if direct else xxn_row[:]
    for m in range(4):
        nc.tensor.matmul(xx_ps[:], mmcast(neghalf[0:C, :]),
                         mmcast(sq[0:C, m * 512:(m + 1) * 512]),
                         start=True, stop=True)
        nc.scalar.activation(
            bass.AP(xx_dst.tensor, xx_dst.offset + m * 512,
                    [xx_dst.ap[0], [1, 512]]),
            xx_ps[:], AF.Copy)
    if aug and not direct:
        nc.sync.dma_start(out=x_rhs[C:C + 1, :], in_=xxn_row[:])

    idx_all = per.tile([P, NT * 24], U32, tag="idx_all")
    v_all = per.tile([P, NT * O], F32, tag="v_all", padded_shape=[P, 16 * 256])

    # ---- phase 0: u/v matmuls for all tiles (independent of selection) ----
    for t in range(NT):
        u_ps = psp.tile([P, O], F32, tag="small_a", padded_shape=[P, 512])
        nc.tensor.matmul(u_ps[:], mmcast(x_aug[0:C, t * P:(t + 1) * P]),
                         mmcast(wn_l[:]), start=True, stop=True)
        u_sb = wrk.tile([P, O], F32, tag="u_sb", padded_shape=[P, 256])
        nc.scalar.activation(u_sb[:], u_ps[:], AF.Copy)
        nc.sync.dma_start(out=u_dram_l[t * P:(t + 1) * P, :], in_=u_sb[:])
        v_ps = psp.tile([P, O], F32, tag="small_b", padded_shape=[P, 512])
        if aug:
            nc.tensor.matmul(v_ps[:], mmcast(x_aug[0:C + 1, t * P:(t + 1) * P]),
                             mmcast(wva[:]), start=True, stop=True)
        else:
            nc.tensor.matmul(v_ps[:], mmcast(x_aug[0:C, t * P:(t + 1) * P]),
                             mmcast(wva[:]), start=True, stop=False)
            nc.tensor.matmul(v_ps[:], mmcast(ones_row[0:1, t * P:(t + 1) * P]),
                             mmcast(wvb[:]), start=False, stop=True)
        nc.scalar.activation(v_all[:, t * O:(t + 1) * O], v_ps[:], AF.Copy)

    # ---- fused: pairwise + selection + gather + combine per row tile ----
    for t in range(NT):
        s_ps = psp.tile([P, N], F32, tag="s")
        lhsT = x_aug[0:C + (1 if aug else 0), t * P:(t + 1) * P]
        for m in range(4):
            rhs = x_rhs[0:C + (1 if aug else 0), m * 512:(m + 1) * 512]
            nc.tensor.matmul(s_ps[:, m * 512:(m + 1) * 512], mmcast(lhsT),
                             mmcast(rhs), start=True, stop=aug)
            if not aug:
                nc.tensor.matmul(s_ps[:, m * 512:(m + 1) * 512],
                                 mmcast(ones_row[0:1, t * P:(t + 1) * P]),
                                 mmcast(xxn_row[0:1, m * 512:(m + 1) * 512]),
                                 start=False, stop=True)
        s_sb = wrk.tile([P, N], F32, tag="big", bufs=3)
        nc.scalar.activation(s_sb[:], s_ps[:], AF.Copy)
        v8 = wrk.tile([P, 8], F32, tag="v8")
        for r in range(3):
            nc.vector.max(v8[:], s_sb[:])
            nc.vector.max_index(idx_all[:, t * 24 + r * 8:t * 24 + r * 8 + 8],
                                v8[:], s_sb[:])
            if r < 2:
                nc.vector.match_replace(s_sb[:], v8[:], s_sb[:], NEG)

        gath = wrk.tile([P, K, O], F32, tag="gath", padded_shape=[P, 20, 256], bufs=3)
        for kk in range(K):
            nc.gpsimd.indirect_dma_start(
                out=gath[:, kk, :], out_offset=None,
                in_=u_dram_l[:],
                in_offset=bass.IndirectOffsetOnAxis(
                    ap=idx_all[:, t * 24 + kk:t * 24 + kk + 1], axis=0))
        xn = wrk.tile([P, O], F32, tag="xn", padded_shape=[P, 256])
        nc.vector.tensor_reduce(out=xn[:], in_=gath[:].rearrange("p k c -> p c k"),
                                axis=AX.X, op=ALU.max)
        nc.vector.tensor_tensor(out=xn[:], in0=xn[:],
                                in1=v_all[:, t * O:(t + 1) * O], op=ALU.add)
        xn2 = wrk.tile([P, O], F32, tag="xn2", padded_shape=[P, 256])
        nc.vector.tensor_scalar(out=xn2[:], in0=xn[:], scalar1=0.2, scalar2=None,
                                op0=ALU.mult)
        nc.vector.tensor_tensor(out=xn[:], in0=xn[:], in1=xn2[:], op=ALU.max)
        for blk in range(O // P if O >= P else 1):
            bw = min(P, O)
            tr_ps = psp2.tile([P, P], F32, tag="tb", padded_shape=[P, 512])
            nc.tensor.transpose(out=tr_ps[0:bw, :],
                                in_=xn[:, blk * P:blk * P + bw],
                                identity=ident_t[:])
            row = catofs + blk * P
            nc.scalar.activation(
                cat_t[row // P][row % P:row % P + bw, t * P:(t + 1) * P],
                tr_ps[0:bw, :], AF.Copy)


def emit_final(nc, tc, wrk, psp2, cloud, out, cat_t, w5_t, b5_t, ones_row,
               mmcast):
    # h = LReLU(W5' cat + b5'); out = [max_n h, mean_n h]
    for eb in range(E5 // P):
        gmax = wrk.tile([P, 4], F32, tag="gmax")
        spre = wrk.tile([P, 4], F32, tag="spre")
        srel = wrk.tile([P, 4], F32, tag="srel")
        for m in range(4):
            h_ps = psp2.tile([P, 512], F32, tag="tb")
            for cc in range(4):
                nc.tensor.matmul(h_ps[:], mmcast(w5_t[cc][:, eb * P:(eb + 1) * P]),
                                 mmcast(cat_t[cc][:, m * 512:(m + 1) * 512]),
                                 start=(cc == 0), stop=False)
            nc.tensor.matmul(h_ps[:], mmcast(b5_t[0:1, eb * P:(eb + 1) * P]),
                             mmcast(ones_row[0:1, m * 512:(m + 1) * 512]),
                             start=False, stop=True)
            nc.vector.tensor_reduce(out=gmax[:, m:m + 1], in_=h_ps[:],
                                    axis=AX.X, op=ALU.max)
            scr = wrk.tile([P, 512], F32, tag="scr")
            nc.scalar.activation(scr[:], h_ps[:], AF.Copy,
                                 accum_out=spre[:, m:m + 1])
            nc.scalar.activation(scr[:], h_ps[:], AF.Relu,
                                 accum_out=srel[:, m:m + 1])
        g1 = wrk.tile([P, 1], F32, tag="g1")
        nc.vector.tensor_reduce(out=g1[:], in_=gmax[:], axis=AX.X, op=ALU.max)
        g2 = wrk.tile([P, 1], F32, tag="g2")
        nc.vector.tensor_scalar(out=g2[:], in0=g1[:], scalar1=0.2, scalar2=None,
                                op0=ALU.mult)
        nc.vector.tensor_tensor(out=g1[:], in0=g1[:], in1=g2[:], op=ALU.max)
        nc.sync.dma_start(out=out[cloud:cloud + 1, eb * P:(eb + 1) * P],
                          in_=g1[:])
        # mean: (0.2*sum(pre) + 0.8*sum(relu)) / N
        sp1 = wrk.tile([P, 1], F32, tag="sp1")
        nc.vector.tensor_reduce(out=sp1[:], in_=spre[:], axis=AX.X, op=ALU.add)
        sr1 = wrk.tile([P, 1], F32, tag="sr1")
        nc.vector.tensor_reduce(out=sr1[:], in_=srel[:], axis=AX.X, op=ALU.add)
        nc.vector.tensor_scalar(out=sp1[:], in0=sp1[:], scalar1=0.2 / N,
                                scalar2=None, op0=ALU.mult)
        nc.vector.tensor_scalar(out=sr1[:], in0=sr1[:], scalar1=0.8 / N,
                                scalar2=None, op0=ALU.mult)
        nc.vector.tensor_tensor(out=sp1[:], in0=sp1[:], in1=sr1[:], op=ALU.add)
        nc.sync.dma_start(out=out[cloud:cloud + 1, E5 + eb * P:E5 + (eb + 1) * P],
                          in_=sp1[:])


_PROGRAM = None


def _get_program():
    global _PROGRAM
    if _PROGRAM is None:
        _PROGRAM = build_program()
    return _PROGRAM


def _fold_bn(p):
    EPS = 1e-5
    W = np.asarray(p['W'], np.float32)
    g, b, m, v = [np.asarray(t, np.float32) for t in p['bn']]
    s = (g / np.sqrt(v + EPS)).astype(np.float32)
    return (W * s[:, None]).astype(np.float32), (b - m * s).astype(np.float32)


def kernel(x, params, _trace=False):
    x = np.asarray(x, np.float32)          # (16, 2048, 3)
    B = x.shape[0]
    nc = _get_program()

    common = {'ident': np.eye(P, dtype=np.float32)}
    for l, key in enumerate(['c1', 'c2', 'c3', 'c4']):
        Wf, bf = _fold_bn(params[key])
        C = Wf.shape[1] // 2
        Wn, Wc = Wf[:, :C], Wf[:, C:]
        common[f'wn{l}'] = np.ascontiguousarray(Wn.T)                  # (C, O)
        wvmat = np.concatenate([(Wc - Wn).T, bf[None, :]], 0)          # (C+1, O)
        common[f'wv{l}'] = np.ascontiguousarray(wvmat)
    W5f, b5f = _fold_bn(params['c5'])
    common['w5'] = np.ascontiguousarray(W5f.T)                         # (512, 1024)
    common['b5'] = np.ascontiguousarray(b5f[None, :])

    in_maps = []
    for core in range(NCORES):
        xc = x[core * BPC:(core + 1) * BPC]                            # (2, 2048, 3)
        xt = np.zeros((BPC, 4, N), np.float32)
        xt[:, 0:3, :] = np.transpose(xc, (0, 2, 1))
        xt[:, 3, :] = 1.0
        m = dict(common)
        m['xin'] = xt
        in_maps.append(m)

    res = bass_utils.run_bass_kernel_spmd(nc, in_maps, list(range(NCORES)),
                                          trace=_trace)
    outs = [res.results[i]['out'] for i in range(NCORES)]              # (2, 2048)
    full = np.concatenate(outs, axis=0).astype(np.float32)             # (16, 2048)
    if _trace:
        return full, res
    return full
